# revision 6
# baseline (speedup 1.0000x reference)
"""Trainium2 Bass kernel for the CLAM-SB style attention-MIL module.

Contract: kernel(**inputs) takes the FULL unsharded inputs (as produced by
setup_inputs()) and returns the full outputs matching reference():
    (logits [1,C], Y_prob [1,C], Y_hat [1], attn_raw [C,N], inst_loss [])

Strategy (8 NeuronCores, shard instance dim N):
  - each core processes N/8 = 8192 rows as 64 tiles of 128
  - host pre-transposes each x shard to x^T [1024, 8192] so the k-contraction
    needs no on-device transpose of x
  - device computes, per shard: h (streamed out), attention scores A,
    exp-sums Z, unnormalized weighted feature sums M_part = sum exp(A)*h,
    and per-partition top-8/bottom-8 candidates per class (vector.max8)
  - host merges: concat attn_raw, softmax stats, global top-k from
    per-shard candidates, h row lookup, tiny instance-classifier / CE tail
"""

import numpy as np

import concourse.bass as bass
import concourse.mybir as mybir
import concourse.tile as tile
from concourse.vector_clock import ScopedClock
from concourse.bass_utils import run_bass_kernel_spmd

# ---------------------------------------------------------------------------
# Problem constants (hardcoded per harness contract)
N, D_IN, D1, D2, C, K = 65536, 1024, 512, 256, 2, 8
N_CORES = 8
P = 128
SHARD = N // N_CORES          # 8192
TILES = SHARD // P            # 64
KC = D_IN // P                # 8 k-chunks for the h matmul
JC = D1 // P                  # 4 d-chunks for the a/g matmul
IC = D2 // P                  # 2 d2-chunks for the A matmul
NSET = 4                      # c0-top, c1-top, c0-bot, c1-bot

f32 = mybir.dt.float32
f32r = mybir.dt.float32r
u32 = mybir.dt.uint32


# ---------------------------------------------------------------------------
# This walrus build rejects instructions carrying more than one embedded
# semaphore wait ("Too many sync wait commands").  Post-pass: hoist excess
# waits onto nofuse NOPs spliced in just before the instruction, same engine.
_MAXW = 1


def _split_embedded_waits(nc):
    cnt = 0
    for fn in nc.m.functions:
        for blk in fn.blocks:
            newlist = []
            for inst in blk.instructions:
                si = getattr(inst, "sync_info", None)
                if si is not None and si.on_wait and len(si.on_wait) > _MAXW:
                    waits = list(si.on_wait)
                    extra, keep = waits[:-_MAXW], waits[-_MAXW:]
                    si.on_wait = keep
                    for w in extra:
                        cnt += 1
                        nop = mybir.InstNoOp(
                            name=f"I-wsplit-{cnt}",
                            engine=inst.engine,
                            bass_nofuse=True,
                            sync_info=mybir.SyncInfo(on_wait=[w], on_update=[]),
                        )
                        newlist.append(nop)
                newlist.append(inst)
            blk.instructions[:] = newlist
    return cnt


# ---------------------------------------------------------------------------
def build_nc(tiles=TILES):
    shard = tiles * P
    nc = bass.Bass()

    # inputs
    xT = nc.declare_dram_parameter("xT", [D_IN, shard], f32r, isOutput=False)
    w1 = nc.declare_dram_parameter("w1", [P, KC, D1], f32r, isOutput=False)
    wab = nc.declare_dram_parameter("wab", [P, JC, D1], f32r, isOutput=False)
    wc = nc.declare_dram_parameter("wc", [P, IC, C], f32r, isOutput=False)
    b1r = nc.declare_dram_parameter("b1r", [P, D1], f32, isOutput=False)
    babr = nc.declare_dram_parameter("babr", [P, D1], f32, isOutput=False)
    ident = nc.declare_dram_parameter("ident", [P, P], f32r, isOutput=False)
    piota = nc.declare_dram_parameter("piota", [P, 1], u32, isOutput=False)

    # outputs
    h_out = nc.declare_dram_parameter("h_out", [shard, D1], f32r, isOutput=True)
    a_out = nc.declare_dram_parameter("a_out", [P, C * tiles], f32, isOutput=True)
    cv_out = nc.declare_dram_parameter("cv_out", [P, NSET * K], f32, isOutput=True)
    ci_out = nc.declare_dram_parameter("ci_out", [P, NSET * K], u32, isOutput=True)
    z_out = nc.declare_dram_parameter("z_out", [P, C], f32, isOutput=True)
    m_out = nc.declare_dram_parameter("m_out", [C, D1], f32, isOutput=True)

    xT_c = xT[:].rearrange("(c k) n -> k c n", k=P)  # [128, KC, shard]

    with tile.TileContext(nc) as tc:
        with (
            tc.tile_pool(name="consts", bufs=1) as consts,
            tc.tile_pool(name="persist", bufs=1) as persist,
            tc.tile_pool(name="xin", bufs=3) as xin,
            tc.tile_pool(name="work", bufs=2) as work,
            tc.tile_pool(name="ph", bufs=2, space="PSUM") as ph,
            tc.tile_pool(name="pag", bufs=2, space="PSUM") as pag,
            tc.tile_pool(name="ptr", bufs=2, space="PSUM") as ptr,
            tc.tile_pool(name="pA", bufs=1, space="PSUM") as pA,
            tc.tile_pool(name="pM", bufs=1, space="PSUM") as pM,
        ):
            # ---- constants into SBUF
            w1_sb = consts.tile([P, KC, D1], f32r)
            nc.sync.dma_start(w1_sb[:], w1[:])
            wab_sb = consts.tile([P, JC, D1], f32r)
            nc.sync.dma_start(wab_sb[:], wab[:])
            wc_sb = consts.tile([P, IC, C], f32r)
            nc.sync.dma_start(wc_sb[:], wc[:])
            b1_sb = consts.tile([P, D1], f32)
            nc.sync.dma_start(b1_sb[:], b1r[:])
            bab_sb = consts.tile([P, D1], f32)
            nc.sync.dma_start(bab_sb[:], babr[:])
            id_sb = consts.tile([P, P], f32r)
            nc.sync.dma_start(id_sb[:], ident[:])
            pi_sb = consts.tile([P, 1], u32)
            nc.sync.dma_start(pi_sb[:], piota[:])

            # ---- persistent accumulators
            A_buf = persist.tile([P, C, tiles], f32)
            E_buf = persist.tile([P, C, tiles], f32)
            m_psum = pM.tile([C, D1], f32)

            for t in range(tiles):
                # load x^T columns for this tile: [128k, KC, 128n]
                xts = xin.tile([P, KC, P], f32r, tag="xts")
                nc.sync.dma_start(xts[:], xT_c[:, :, t * P:(t + 1) * P])

                # h = x @ W1  -> [128n, 512]
                h_psum = ph.tile([P, D1], f32, tag="h")
                for c in range(KC):
                    nc.tensor.matmul(
                        h_psum[:],
                        xts[:, c, :],
                        w1_sb[:, c, :],
                        start=(c == 0),
                        stop=(c == KC - 1),
                    )
                # + b1 (free-dim bias) on DVE, relu on ACT
                hb_sb = work.tile([P, D1], f32, tag="hb")
                nc.vector.tensor_add(hb_sb[:], h_psum[:], b1_sb[:])
                h_sb = work.tile([P, D1], f32r, tag="h")
                nc.scalar.activation(
                    h_sb[:], hb_sb[:], mybir.ActivationFunctionType.Relu
                )
                # stream h out to HBM
                nc.sync.dma_start(h_out[t * P:(t + 1) * P, :], h_sb[:])

                # h^T via PE transpose (4 chunks of 128)
                tr1 = ptr.tile([P, D1], f32r, tag="tr")
                for j in range(JC):
                    nc.tensor.transpose(
                        tr1[:, j * P:(j + 1) * P],
                        h_sb[:, j * P:(j + 1) * P],
                        id_sb[:],
                    )
                hT_sb = work.tile([P, JC, P], f32r, tag="hT")
                nc.vector.tensor_copy(
                    hT_sb[:].rearrange("p j n -> p (j n)"), tr1[:]
                )

                # [a|g] = h @ [Wa|Wb]  -> [128n, 512]
                ag_psum = pag.tile([P, D1], f32, tag="ag")
                for j in range(JC):
                    nc.tensor.matmul(
                        ag_psum[:],
                        hT_sb[:, j, :],
                        wab_sb[:, j, :],
                        start=(j == 0),
                        stop=(j == JC - 1),
                    )
                agb_sb = work.tile([P, D1], f32, tag="agb")
                nc.vector.tensor_add(agb_sb[:], ag_psum[:], bab_sb[:])
                ta_sb = work.tile([P, D1], f32, tag="ta")
                nc.scalar.activation(
                    ta_sb[:, :D2], agb_sb[:, :D2],
                    mybir.ActivationFunctionType.Tanh,
                )
                nc.scalar.activation(
                    ta_sb[:, D2:], agb_sb[:, D2:],
                    mybir.ActivationFunctionType.Sigmoid,
                )
                ag_sb = work.tile([P, D2], f32r, tag="agm")
                nc.vector.tensor_mul(ag_sb[:], ta_sb[:, :D2], ta_sb[:, D2:])

                # ag^T via PE transpose (2 chunks)
                tr2 = ptr.tile([P, D1], f32r, tag="tr")
                for i in range(IC):
                    nc.tensor.transpose(
                        tr2[:, i * P:(i + 1) * P],
                        ag_sb[:, i * P:(i + 1) * P],
                        id_sb[:],
                    )
                agT_sb = work.tile([P, IC, P], f32r, tag="agT")
                nc.vector.tensor_copy(
                    agT_sb[:].rearrange("p i n -> p (i n)"), tr2[:, :D2]
                )

                # A = ag @ Wc  -> [128n, 2]   (bc added host-side)
                A_psum = pA.tile([P, C], f32, tag="A")
                for i in range(IC):
                    nc.tensor.matmul(
                        A_psum[:],
                        agT_sb[:, i, :],
                        wc_sb[:, i, :],
                        start=(i == 0),
                        stop=(i == IC - 1),
                    )
                nc.vector.tensor_copy(A_buf[:, :, t], A_psum[:])

                # E = exp(A); contiguous copy for the matmul + strided for Z
                E_tile = work.tile([P, C], f32r, tag="E")
                nc.scalar.activation(
                    E_tile[:], A_psum[:], mybir.ActivationFunctionType.Exp
                )
                nc.scalar.activation(
                    E_buf[:, :, t], A_psum[:], mybir.ActivationFunctionType.Exp
                )

                # M_part += E^T @ h  (accumulate [2, 512] across all tiles)
                nc.tensor.matmul(
                    m_psum[:],
                    E_tile[:],
                    h_sb[:],
                    start=(t == 0),
                    stop=(t == tiles - 1),
                    skip_group_check=True,
                )

            # ---- tail: stats + per-partition top-k candidates
            z_sb = persist.tile([P, C], f32)
            nc.vector.tensor_reduce(
                z_sb[:], E_buf[:], axis=mybir.AxisListType.X,
                op=mybir.AluOpType.add,
            )
            nc.sync.dma_start(z_out[:], z_sb[:])

            m_sb = persist.tile([C, D1], f32)
            nc.vector.tensor_copy(m_sb[:], m_psum[:])
            nc.sync.dma_start(m_out[:], m_sb[:])

            nc.sync.dma_start(a_out[:], A_buf[:].rearrange("p c t -> p (c t)"))

            negA = persist.tile([P, C, tiles], f32)
            nc.vector.tensor_scalar_mul(
                negA[:].rearrange("p c t -> p (c t)"),
                A_buf[:].rearrange("p c t -> p (c t)"),
                -1.0,
            )

            cv_sb = persist.tile([P, NSET * K], f32)
            ti_sb = persist.tile([P, NSET * K], u32)
            ci_sb = persist.tile([P, NSET * K], u32)
            sets = [A_buf[:, 0, :], A_buf[:, 1, :], negA[:, 0, :], negA[:, 1, :]]
            for s, ap in enumerate(sets):
                nc.vector.max(cv_sb[:, s * K:(s + 1) * K], ap)
                nc.vector.max_index(
                    ti_sb[:, s * K:(s + 1) * K], cv_sb[:, s * K:(s + 1) * K], ap
                )
            # n_local = t_idx * 128 + partition
            nc.vector.tensor_scalar(
                ci_sb[:], ti_sb[:], P, scalar2=None, op0=mybir.AluOpType.mult
            )
            nc.vector.tensor_add(
                ci_sb[:], ci_sb[:], pi_sb[:].to_broadcast([P, NSET * K])
            )
            nc.sync.dma_start(cv_out[:], cv_sb[:])
            nc.sync.dma_start(ci_out[:], ci_sb[:])

    return nc


# ---------------------------------------------------------------------------
_CACHE = {}


def _get_nc(tiles):
    if tiles not in _CACHE:
        nc = build_nc(tiles)
        _split_embedded_waits(nc)
        _CACHE[tiles] = nc
    return _CACHE[tiles]


def _prep_consts(W1, b1, Wa, ba, Wb, bb, Wc, bc):
    f = np.float32
    w1 = np.ascontiguousarray(
        W1.astype(f).reshape(KC, P, D1).transpose(1, 0, 2)
    )
    wab_full = np.concatenate([Wa.astype(f), Wb.astype(f)], axis=1)  # [512,512]
    wab = np.ascontiguousarray(wab_full.reshape(JC, P, D1).transpose(1, 0, 2))
    wc = np.ascontiguousarray(Wc.astype(f).reshape(IC, P, C).transpose(1, 0, 2))
    b1r = np.ascontiguousarray(np.broadcast_to(b1.astype(f), (P, D1)))
    bab_full = np.concatenate([ba.astype(f), bb.astype(f)])  # [512]
    babr = np.ascontiguousarray(np.broadcast_to(bab_full, (P, D1)))
    ident = np.eye(P, dtype=f)
    piota = np.arange(P, dtype=np.uint32).reshape(P, 1)
    return dict(w1=w1, wab=wab, wc=wc, b1r=b1r, babr=babr, ident=ident,
                piota=piota)


def run_device(x, W1, b1, Wa, ba, Wb, bb, Wc, bc, tiles=TILES, n_cores=N_CORES):
    """Run the device program; returns per-core result dicts."""
    nc = _get_nc(tiles)
    consts = _prep_consts(W1, b1, Wa, ba, Wb, bb, Wc, bc)
    shard = tiles * P
    in_maps = []
    for s in range(n_cores):
        xs = x[s * shard:(s + 1) * shard].astype(np.float32)
        in_maps.append(dict(consts, xT=np.ascontiguousarray(xs.T)))
    res = run_bass_kernel_spmd(nc, in_maps, core_ids=list(range(n_cores)))
    return res.results


def _log_softmax(x, axis=-1):
    m = np.max(x, axis=axis, keepdims=True)
    s = x - m
    return s - np.log(np.sum(np.exp(s), axis=axis, keepdims=True))


def _topk_from_candidates(vals, idxs, k):
    """Exact top-k (value desc, index asc tiebreak) from candidate pool."""
    order = np.lexsort((idxs, -vals))
    sel = order[:k]
    return vals[sel], idxs[sel]


def kernel(x, target, W1, b1, Wa, ba, Wb, bb, Wc, bc, Wcls, bcls, Wins, bins):
    x = np.asarray(x)
    results = run_device(np.asarray(x), np.asarray(W1), np.asarray(b1),
                         np.asarray(Wa), np.asarray(ba), np.asarray(Wb),
                         np.asarray(bb), np.asarray(Wc), np.asarray(bc))

    f = np.float32
    bc = np.asarray(bc).astype(f)

    # ---- assemble attn_raw [C, N] (device A + host-side bc)
    attn_parts = []
    h_parts = []
    Z = np.zeros(C, dtype=np.float64)
    M_us = np.zeros((C, D1), dtype=np.float64)
    cand_v = {s: [] for s in range(NSET)}
    cand_i = {s: [] for s in range(NSET)}
    for s_core, r in enumerate(results):
        A_shard = r["a_out"].reshape(P, C, TILES).transpose(1, 2, 0).reshape(C, SHARD)
        attn_parts.append(A_shard)
        h_parts.append(r["h_out"])
        Z += r["z_out"].astype(np.float64).sum(axis=0)
        M_us += r["m_out"].astype(np.float64)
        cv = r["cv_out"]          # [P, 32]
        ci = r["ci_out"].astype(np.int64) + s_core * SHARD
        for st in range(NSET):
            cand_v[st].append(cv[:, st * K:(st + 1) * K].ravel())
            cand_i[st].append(ci[:, st * K:(st + 1) * K].ravel())

    A_nobc = np.concatenate(attn_parts, axis=1)            # [C, N]
    attn_raw = (A_nobc + bc[:, None]).astype(f)
    h_full = np.concatenate(h_parts, axis=0)               # [N, D1]

    # ---- softmax-weighted slide features M = (softmax(A) @ h)
    M = (M_us / Z[:, None]).astype(f)                      # [C, D1]

    # ---- bag-level classifier
    Wcls = np.asarray(Wcls).astype(f)
    bcls = np.asarray(bcls).astype(f)
    logits = (M @ Wcls + bcls).T.astype(f)                 # [1, C]
    e = np.exp(logits - logits.max(axis=1, keepdims=True))
    Y_prob = (e / e.sum(axis=1, keepdims=True)).astype(f)
    Y_hat = np.argmax(logits, axis=1).astype(np.int32)

    # ---- instance branch: exact global top/bottom-k from shard candidates
    top_ids = np.zeros((C, K), dtype=np.int64)
    bot_ids = np.zeros((C, K), dtype=np.int64)
    for c in range(C):
        tv = np.concatenate(cand_v[c])
        ti = np.concatenate(cand_i[c])
        _, top_ids[c] = _topk_from_candidates(tv, ti, K)
        bv = np.concatenate(cand_v[C + c])
        bi = np.concatenate(cand_i[C + c])
        _, bot_ids[c] = _topk_from_candidates(bv, bi, K)

    top_p = h_full[top_ids]                                # [C, K, D1]
    top_n = h_full[bot_ids]                                # [C, K, D1]

    Wins = np.asarray(Wins).astype(f)                      # [C, D1, 2]
    bins = np.asarray(bins).astype(f)                      # [C, 2]
    inst_in = np.concatenate([top_p, top_n], axis=1)       # [C, 2K, D1]
    logits_in = np.einsum("ckd,cdo->cko", inst_in, Wins) + bins[:, None, :]
    logp_in = _log_softmax(logits_in, axis=-1)             # [C, 2K, 2]
    tgt_in = np.concatenate([np.ones(K, np.int64), np.zeros(K, np.int64)])
    sel_in = np.take_along_axis(
        logp_in, np.broadcast_to(tgt_in[None, :, None], (C, 2 * K, 1)), axis=2
    )[..., 0]
    loss_in = -np.mean(sel_in, axis=1)                     # [C]

    logits_out = np.einsum("ckd,cdo->cko", top_p, Wins) + bins[:, None, :]
    logp_out = _log_softmax(logits_out, axis=-1)
    loss_out = -np.mean(logp_out[..., 0], axis=1)          # [C]

    tgt = int(np.asarray(target))
    inst_labels = np.zeros(C, dtype=np.int64)
    inst_labels[tgt] = 1
    inst_loss = np.mean(
        np.where(inst_labels == 1, loss_in, loss_out)
    ).astype(f)

    return (logits, Y_prob, Y_hat, attn_raw, np.float32(inst_loss))


# revision 10
# speedup vs baseline: 50851.5842x; 50851.5842x over previous
"""Trainium2 Bass kernel for the CLAM-SB style attention-MIL module.

Contract: kernel(**inputs) takes the FULL unsharded inputs (as produced by
setup_inputs()) and returns the full outputs matching reference():
    (logits [1,C], Y_prob [1,C], Y_hat [1], attn_raw [C,N], inst_loss [])

Strategy (8 NeuronCores, shard instance dim N):
  - each core processes N/8 = 8192 rows as 64 tiles of 128
  - host pre-transposes each x shard to x^T [1024, 8192] so the k-contraction
    needs no on-device transpose of x
  - device computes, per shard: h (streamed out), attention scores A,
    exp-sums Z, unnormalized weighted feature sums M_part = sum exp(A)*h,
    and per-partition top-8/bottom-8 candidates per class (vector.max8)
  - host merges: concat attn_raw, softmax stats, global top-k from
    per-shard candidates, h row lookup, tiny instance-classifier / CE tail
"""

import numpy as np

import concourse.bass as bass
import concourse.mybir as mybir
import concourse.tile as tile
from concourse.vector_clock import ScopedClock
from concourse.bass_utils import run_bass_kernel_spmd

# ---------------------------------------------------------------------------
# Problem constants (hardcoded per harness contract)
N, D_IN, D1, D2, C, K = 65536, 1024, 512, 256, 2, 8
N_CORES = 8
P = 128
SHARD = N // N_CORES          # 8192
TILES = SHARD // P            # 64
KC = D_IN // P                # 8 k-chunks for the h matmul
JC = D1 // P                  # 4 d-chunks for the a/g matmul
IC = D2 // P                  # 2 d2-chunks for the A matmul
NSET = 4                      # c0-top, c1-top, c0-bot, c1-bot

f32 = mybir.dt.float32
f32r = mybir.dt.float32r
u32 = mybir.dt.uint32


# ---------------------------------------------------------------------------
# This walrus build rejects instructions carrying more than one embedded
# semaphore wait ("Too many sync wait commands").  Post-pass: hoist excess
# waits onto nofuse NOPs spliced in just before the instruction, same engine.
_MAXW = 1


def _split_embedded_waits(nc):
    cnt = 0
    for fn in nc.m.functions:
        for blk in fn.blocks:
            newlist = []
            for inst in blk.instructions:
                si = getattr(inst, "sync_info", None)
                if si is not None and si.on_wait and len(si.on_wait) > _MAXW:
                    waits = list(si.on_wait)
                    extra, keep = waits[:-_MAXW], waits[-_MAXW:]
                    si.on_wait = keep
                    for w in extra:
                        cnt += 1
                        nop = mybir.InstNoOp(
                            name=f"I-wsplit-{cnt}",
                            engine=inst.engine,
                            bass_nofuse=True,
                            sync_info=mybir.SyncInfo(on_wait=[w], on_update=[]),
                        )
                        newlist.append(nop)
                newlist.append(inst)
            blk.instructions[:] = newlist
    return cnt


# ---------------------------------------------------------------------------
def build_nc(tiles=TILES, reps=1):
    shard = tiles * P
    nc = bass.Bass()

    # inputs
    xT = nc.declare_dram_parameter("xT", [D_IN, shard], f32r, isOutput=False)
    w1 = nc.declare_dram_parameter("w1", [P, KC, D1], f32r, isOutput=False)
    wab = nc.declare_dram_parameter("wab", [P, JC, D1], f32r, isOutput=False)
    wc = nc.declare_dram_parameter("wc", [P, IC, C], f32r, isOutput=False)
    b1r = nc.declare_dram_parameter("b1r", [P, D1], f32, isOutput=False)
    babr = nc.declare_dram_parameter("babr", [P, D1], f32, isOutput=False)
    ident = nc.declare_dram_parameter("ident", [P, P], f32r, isOutput=False)
    piota = nc.declare_dram_parameter("piota", [P, 1], u32, isOutput=False)

    # outputs
    h_out = nc.declare_dram_parameter("h_out", [shard, D1], f32r, isOutput=True)
    a_out = nc.declare_dram_parameter("a_out", [P, C * tiles], f32, isOutput=True)
    cv_out = nc.declare_dram_parameter("cv_out", [P, NSET * K], f32, isOutput=True)
    ci_out = nc.declare_dram_parameter("ci_out", [P, NSET * K], u32, isOutput=True)
    z_out = nc.declare_dram_parameter("z_out", [P, C], f32, isOutput=True)
    m_out = nc.declare_dram_parameter("m_out", [C, D1], f32, isOutput=True)

    xT_c = xT[:].rearrange("(c k) n -> k c n", k=P)  # [128, KC, shard]

    with tile.TileContext(nc) as tc:
        with (
            tc.tile_pool(name="consts", bufs=1) as consts,
            tc.tile_pool(name="persist", bufs=1) as persist,
            tc.tile_pool(name="xin", bufs=3) as xin,
            tc.tile_pool(name="work", bufs=2) as work,
            tc.tile_pool(name="ph", bufs=2, space="PSUM") as ph,
            tc.tile_pool(name="pag", bufs=2, space="PSUM") as pag,
            tc.tile_pool(name="ptr", bufs=2, space="PSUM") as ptr,
            tc.tile_pool(name="pA", bufs=1, space="PSUM") as pA,
            tc.tile_pool(name="pM", bufs=1, space="PSUM") as pM,
        ):
            # ---- constants into SBUF
            w1_sb = consts.tile([P, KC, D1], f32r)
            nc.sync.dma_start(w1_sb[:], w1[:])
            wab_sb = consts.tile([P, JC, D1], f32r)
            nc.sync.dma_start(wab_sb[:], wab[:])
            wc_sb = consts.tile([P, IC, C], f32r)
            nc.sync.dma_start(wc_sb[:], wc[:])
            b1_sb = consts.tile([P, D1], f32)
            nc.sync.dma_start(b1_sb[:], b1r[:])
            bab_sb = consts.tile([P, D1], f32)
            nc.sync.dma_start(bab_sb[:], babr[:])
            id_sb = consts.tile([P, P], f32r)
            nc.sync.dma_start(id_sb[:], ident[:])
            pi_sb = consts.tile([P, 1], u32)
            nc.sync.dma_start(pi_sb[:], piota[:])

            # ---- persistent accumulators
            A_buf = persist.tile([P, C, tiles], f32)
            E_buf = persist.tile([P, C, tiles], f32)
            m_psum = pM.tile([C, D1], f32)

            # Software-pipelined emission: PE executes instructions in
            # program order, so interleaving same-tile dependent stages
            # stalls PE on DVE/ACT round-trips.  Skew: iteration i emits
            # stage S_k for logical tile i-k, giving each cross-engine hop
            # a full iteration of slack.
            st = {}  # logical tile -> dict of live tiles

            def s0_load(t):
                xts = xin.tile([P, KC, P], f32r, tag="xts")
                tm = t % tiles
                nc.sync.dma_start(xts[:], xT_c[:, :, tm * P:(tm + 1) * P])
                st[t] = {"xts": xts}

            def s1_hmm(t):
                d = st[t]
                h_psum = ph.tile([P, D1], f32, tag="h")
                for c in range(KC):
                    nc.tensor.matmul(
                        h_psum[:],
                        d["xts"][:, c, :],
                        w1_sb[:, c, :],
                        start=(c == 0),
                        stop=(c == KC - 1),
                    )
                hb_sb = work.tile([P, D1], f32, tag="hb")
                nc.vector.tensor_add(hb_sb[:], h_psum[:], b1_sb[:])
                h_sb = work.tile([P, D1], f32r, tag="h", bufs=7)
                nc.scalar.activation(
                    h_sb[:], hb_sb[:], mybir.ActivationFunctionType.Relu
                )
                tm = t % tiles
                nc.sync.dma_start(h_out[tm * P:(tm + 1) * P, :], h_sb[:])
                d["h_sb"] = h_sb

            def s2_trh(t):
                d = st[t]
                tr1 = ptr.tile([P, D1], f32r, tag="tr")
                for j in range(JC):
                    nc.tensor.transpose(
                        tr1[:, j * P:(j + 1) * P],
                        d["h_sb"][:, j * P:(j + 1) * P],
                        id_sb[:],
                    )
                hT_sb = work.tile([P, JC, P], f32r, tag="hT", bufs=3)
                nc.vector.tensor_copy(
                    hT_sb[:].rearrange("p j n -> p (j n)"), tr1[:]
                )
                d["hT_sb"] = hT_sb

            def s3_agmm(t):
                d = st[t]
                ag_psum = pag.tile([P, D1], f32, tag="ag")
                for j in range(JC):
                    nc.tensor.matmul(
                        ag_psum[:],
                        d["hT_sb"][:, j, :],
                        wab_sb[:, j, :],
                        start=(j == 0),
                        stop=(j == JC - 1),
                    )
                agb_sb = work.tile([P, D1], f32, tag="agb")
                nc.vector.tensor_add(agb_sb[:], ag_psum[:], bab_sb[:])
                ta_sb = work.tile([P, D1], f32, tag="ta")
                nc.scalar.activation(
                    ta_sb[:, :D2], agb_sb[:, :D2],
                    mybir.ActivationFunctionType.Tanh,
                )
                nc.scalar.activation(
                    ta_sb[:, D2:], agb_sb[:, D2:],
                    mybir.ActivationFunctionType.Sigmoid,
                )
                ag_sb = work.tile([P, D2], f32r, tag="agm", bufs=3)
                nc.vector.tensor_mul(ag_sb[:], ta_sb[:, :D2], ta_sb[:, D2:])
                d["ag_sb"] = ag_sb

            def s4_trag(t):
                d = st[t]
                tr2 = ptr.tile([P, D1], f32r, tag="tr")
                for i in range(IC):
                    nc.tensor.transpose(
                        tr2[:, i * P:(i + 1) * P],
                        d["ag_sb"][:, i * P:(i + 1) * P],
                        id_sb[:],
                    )
                agT_sb = work.tile([P, IC, P], f32r, tag="agT", bufs=3)
                nc.vector.tensor_copy(
                    agT_sb[:].rearrange("p i n -> p (i n)"), tr2[:, :D2]
                )
                d["agT_sb"] = agT_sb

            def s5_amm(t):
                d = st[t]
                A_psum = pA.tile([P, C], f32, tag="A")
                for i in range(IC):
                    nc.tensor.matmul(
                        A_psum[:],
                        d["agT_sb"][:, i, :],
                        wc_sb[:, i, :],
                        start=(i == 0),
                        stop=(i == IC - 1),
                    )
                nc.vector.tensor_copy(A_buf[:, :, t % tiles], A_psum[:])
                E_tile = work.tile([P, C], f32r, tag="E", bufs=3)
                nc.scalar.activation(
                    E_tile[:], A_psum[:], mybir.ActivationFunctionType.Exp
                )
                nc.scalar.activation(
                    E_buf[:, :, t % tiles], A_psum[:],
                    mybir.ActivationFunctionType.Exp,
                )
                d["E_tile"] = E_tile

            def s6_mmm(t):
                d = st[t]
                nc.tensor.matmul(
                    m_psum[:],
                    d["E_tile"][:],
                    d["h_sb"][:],
                    start=(t % tiles == 0),
                    stop=(t % tiles == tiles - 1),
                    skip_group_check=True,
                )
                del st[t]

            stages = [s1_hmm, s2_trh, s3_agmm, s4_trag, s5_amm, s6_mmm]
            SKEW = len(stages)  # s0 prefetch distance 2 ahead of s1
            total = reps * tiles
            for i in range(total + SKEW):
                if i < total:
                    s0_load(i)
                for k, fn_stage in enumerate(stages):
                    t = i - 1 - k
                    if 0 <= t < total:
                        fn_stage(t)

            # ---- tail: stats + per-partition top-k candidates
            z_sb = persist.tile([P, C], f32)
            nc.vector.tensor_reduce(
                z_sb[:], E_buf[:], axis=mybir.AxisListType.X,
                op=mybir.AluOpType.add,
            )
            nc.sync.dma_start(z_out[:], z_sb[:])

            m_sb = persist.tile([C, D1], f32)
            nc.vector.tensor_copy(m_sb[:], m_psum[:])
            nc.sync.dma_start(m_out[:], m_sb[:])

            nc.sync.dma_start(a_out[:], A_buf[:].rearrange("p c t -> p (c t)"))

            negA = persist.tile([P, C, tiles], f32)
            nc.vector.tensor_scalar_mul(
                negA[:].rearrange("p c t -> p (c t)"),
                A_buf[:].rearrange("p c t -> p (c t)"),
                -1.0,
            )

            cv_sb = persist.tile([P, NSET * K], f32)
            ti_sb = persist.tile([P, NSET * K], u32)
            ci_sb = persist.tile([P, NSET * K], u32)
            sets = [A_buf[:, 0, :], A_buf[:, 1, :], negA[:, 0, :], negA[:, 1, :]]
            for s, ap in enumerate(sets):
                nc.vector.max(cv_sb[:, s * K:(s + 1) * K], ap)
                nc.vector.max_index(
                    ti_sb[:, s * K:(s + 1) * K], cv_sb[:, s * K:(s + 1) * K], ap
                )
            # n_local = t_idx * 128 + partition
            nc.vector.tensor_scalar(
                ci_sb[:], ti_sb[:], P, scalar2=None, op0=mybir.AluOpType.mult
            )
            nc.vector.tensor_add(
                ci_sb[:], ci_sb[:], pi_sb[:].to_broadcast([P, NSET * K])
            )
            nc.sync.dma_start(cv_out[:], cv_sb[:])
            nc.sync.dma_start(ci_out[:], ci_sb[:])

    return nc


# ---------------------------------------------------------------------------
_CACHE = {}


def _get_nc(tiles, reps=1):
    key = (tiles, reps)
    if key not in _CACHE:
        nc = build_nc(tiles, reps)
        _split_embedded_waits(nc)
        _CACHE[key] = nc
    return _CACHE[key]


def _prep_consts(W1, b1, Wa, ba, Wb, bb, Wc, bc):
    f = np.float32
    w1 = np.ascontiguousarray(
        W1.astype(f).reshape(KC, P, D1).transpose(1, 0, 2)
    )
    wab_full = np.concatenate([Wa.astype(f), Wb.astype(f)], axis=1)  # [512,512]
    wab = np.ascontiguousarray(wab_full.reshape(JC, P, D1).transpose(1, 0, 2))
    wc = np.ascontiguousarray(Wc.astype(f).reshape(IC, P, C).transpose(1, 0, 2))
    b1r = np.ascontiguousarray(np.broadcast_to(b1.astype(f), (P, D1)))
    bab_full = np.concatenate([ba.astype(f), bb.astype(f)])  # [512]
    babr = np.ascontiguousarray(np.broadcast_to(bab_full, (P, D1)))
    ident = np.eye(P, dtype=f)
    piota = np.arange(P, dtype=np.uint32).reshape(P, 1)
    return dict(w1=w1, wab=wab, wc=wc, b1r=b1r, babr=babr, ident=ident,
                piota=piota)


def run_device(x, W1, b1, Wa, ba, Wb, bb, Wc, bc, tiles=TILES, n_cores=N_CORES):
    """Run the device program; returns per-core result dicts."""
    nc = _get_nc(tiles)
    consts = _prep_consts(W1, b1, Wa, ba, Wb, bb, Wc, bc)
    shard = tiles * P
    in_maps = []
    for s in range(n_cores):
        xs = x[s * shard:(s + 1) * shard].astype(np.float32)
        in_maps.append(dict(consts, xT=np.ascontiguousarray(xs.T)))
    res = run_bass_kernel_spmd(nc, in_maps, core_ids=list(range(n_cores)))
    return res.results


def _log_softmax(x, axis=-1):
    m = np.max(x, axis=axis, keepdims=True)
    s = x - m
    return s - np.log(np.sum(np.exp(s), axis=axis, keepdims=True))


def _topk_from_candidates(vals, idxs, k):
    """Exact top-k (value desc, index asc tiebreak) from candidate pool."""
    order = np.lexsort((idxs, -vals))
    sel = order[:k]
    return vals[sel], idxs[sel]


def kernel(x, target, W1, b1, Wa, ba, Wb, bb, Wc, bc, Wcls, bcls, Wins, bins):
    x = np.asarray(x)
    results = run_device(np.asarray(x), np.asarray(W1), np.asarray(b1),
                         np.asarray(Wa), np.asarray(ba), np.asarray(Wb),
                         np.asarray(bb), np.asarray(Wc), np.asarray(bc))

    f = np.float32
    bc = np.asarray(bc).astype(f)

    # ---- assemble attn_raw [C, N] (device A + host-side bc)
    attn_parts = []
    h_parts = []
    Z = np.zeros(C, dtype=np.float64)
    M_us = np.zeros((C, D1), dtype=np.float64)
    cand_v = {s: [] for s in range(NSET)}
    cand_i = {s: [] for s in range(NSET)}
    for s_core, r in enumerate(results):
        A_shard = r["a_out"].reshape(P, C, TILES).transpose(1, 2, 0).reshape(C, SHARD)
        attn_parts.append(A_shard)
        h_parts.append(r["h_out"])
        Z += r["z_out"].astype(np.float64).sum(axis=0)
        M_us += r["m_out"].astype(np.float64)
        cv = r["cv_out"]          # [P, 32]
        ci = r["ci_out"].astype(np.int64) + s_core * SHARD
        for st in range(NSET):
            cand_v[st].append(cv[:, st * K:(st + 1) * K].ravel())
            cand_i[st].append(ci[:, st * K:(st + 1) * K].ravel())

    A_nobc = np.concatenate(attn_parts, axis=1)            # [C, N]
    attn_raw = (A_nobc + bc[:, None]).astype(f)
    h_full = np.concatenate(h_parts, axis=0)               # [N, D1]

    # ---- softmax-weighted slide features M = (softmax(A) @ h)
    M = (M_us / Z[:, None]).astype(f)                      # [C, D1]

    # ---- bag-level classifier
    Wcls = np.asarray(Wcls).astype(f)
    bcls = np.asarray(bcls).astype(f)
    logits = (M @ Wcls + bcls).T.astype(f)                 # [1, C]
    e = np.exp(logits - logits.max(axis=1, keepdims=True))
    Y_prob = (e / e.sum(axis=1, keepdims=True)).astype(f)
    Y_hat = np.argmax(logits, axis=1).astype(np.int32)

    # ---- instance branch: exact global top/bottom-k from shard candidates
    top_ids = np.zeros((C, K), dtype=np.int64)
    bot_ids = np.zeros((C, K), dtype=np.int64)
    for c in range(C):
        tv = np.concatenate(cand_v[c])
        ti = np.concatenate(cand_i[c])
        _, top_ids[c] = _topk_from_candidates(tv, ti, K)
        bv = np.concatenate(cand_v[C + c])
        bi = np.concatenate(cand_i[C + c])
        _, bot_ids[c] = _topk_from_candidates(bv, bi, K)

    top_p = h_full[top_ids]                                # [C, K, D1]
    top_n = h_full[bot_ids]                                # [C, K, D1]

    Wins = np.asarray(Wins).astype(f)                      # [C, D1, 2]
    bins = np.asarray(bins).astype(f)                      # [C, 2]
    inst_in = np.concatenate([top_p, top_n], axis=1)       # [C, 2K, D1]
    logits_in = np.einsum("ckd,cdo->cko", inst_in, Wins) + bins[:, None, :]
    logp_in = _log_softmax(logits_in, axis=-1)             # [C, 2K, 2]
    tgt_in = np.concatenate([np.ones(K, np.int64), np.zeros(K, np.int64)])
    sel_in = np.take_along_axis(
        logp_in, np.broadcast_to(tgt_in[None, :, None], (C, 2 * K, 1)), axis=2
    )[..., 0]
    loss_in = -np.mean(sel_in, axis=1)                     # [C]

    logits_out = np.einsum("ckd,cdo->cko", top_p, Wins) + bins[:, None, :]
    logp_out = _log_softmax(logits_out, axis=-1)
    loss_out = -np.mean(logp_out[..., 0], axis=1)          # [C]

    tgt = int(np.asarray(target))
    inst_labels = np.zeros(C, dtype=np.int64)
    inst_labels[tgt] = 1
    inst_loss = np.mean(
        np.where(inst_labels == 1, loss_in, loss_out)
    ).astype(f)

    return (logits, Y_prob, Y_hat, attn_raw, np.float32(inst_loss))
